# revision 1
# baseline (speedup 1.0000x reference)
"""Causal self-attention (SEQ=8192, D=1024) on 8 TRN2 NeuronCores.

Strategy (SPMD, one static graph on all 8 cores):
  - Sequence parallel over queries with stride-8 row interleaving:
    core i owns query rows {8j+i : j in [0,1024)}. This balances causal
    work exactly while keeping the instruction graph identical across
    cores (per-core differences are pure data: X^T slices + masks).
  - Core i computes K^T/V projections for the contiguous key shard
    [1024*i, 1024*(i+1)). K^T/V are shared via FOUR chunked AllGathers
    (K/V x key-halves), each issued as soon as its projection slice is
    done, so the collectives overlap projection + attention compute.
  - Attention runs in S^T layout ([keys x queries]): S^T = K^T.T @ Q^T,
    so softmax(P)^T is directly the lhsT for P@V -- no transposes.
    It is split into two passes over key-halves; pass 0 only needs the
    first two gathered chunks. exp on ScalarE (scale fused), no
    max-subtraction (scores are N(0,1)-scaled), denominator via a
    ones-column matmul accumulated alongside O in PSUM.
  - All matmul operands bf16 (1 cyc/row on the PE), accumulation fp32.
"""
import sys

sys.path.insert(0, "/opt/trn_rl_repo")

import numpy as np
import ml_dtypes

import concourse.bacc as bacc
import concourse.mybir as mybir
import concourse.tile as tile
from concourse import bass_utils

S, D, NC = 8192, 1024, 8
QPC = S // NC  # 1024 queries (and kv rows) per core
NCH = D // 128  # 8 chunks of the feature dim
NQT = QPC // 128  # 8 query tiles per core
SCALE = 1.0 / np.sqrt(D).astype(np.float32)  # 1/32
BF16 = mybir.dt.bfloat16
F32 = mybir.dt.float32

_cache = {}


def _build():
    if "nc" in _cache:
        return _cache["nc"]
    nc = bacc.Bacc("TRN2", target_bir_lowering=False, debug=False, num_devices=NC)

    xt_kv = nc.dram_tensor("xt_kv", [D, QPC], BF16, kind="ExternalInput")
    xt_q = nc.dram_tensor("xt_q", [D, QPC], BF16, kind="ExternalInput")
    wkT = nc.dram_tensor("wkT", [D, D], BF16, kind="ExternalInput")
    wvT = nc.dram_tensor("wvT", [D, D], BF16, kind="ExternalInput")
    masks = nc.dram_tensor("masks", [8, 128, 128], BF16, kind="ExternalInput")
    out = nc.dram_tensor("out", [QPC, D], F32, kind="ExternalOutput")

    rg = [list(range(NC))]

    with tile.TileContext(nc) as tc:
        with tc.tile_pool(name="dram", bufs=1, space="DRAM") as dram:
            # chunked AllGather bounce buffers: K^T key-halves, V key-halves
            ag_k = [dram.tile([D, 512], BF16, name=f"agk{h}") for h in range(2)]
            ag_v = [dram.tile([512, D], BF16, name=f"agv{h}") for h in range(2)]
            g_k = [
                dram.tile([NC, D, 512], BF16, addr_space="Shared", name=f"gk{h}")
                for h in range(2)
            ]
            g_v = [
                dram.tile([NC, 512, D], BF16, addr_space="Shared", name=f"gv{h}")
                for h in range(2)
            ]

            with (
                tc.tile_pool(name="persist", bufs=1) as persist,
                tc.tile_pool(name="fin", bufs=2) as fin,
            ):
                sb_qt = persist.tile([128, NCH * QPC], BF16, tag="qt")
                sb_mask = persist.tile([128, 8 * 128], BF16, tag="msk")
                sb_ones = persist.tile([128, 1], BF16, tag="ones")
                nc.vector.memset(sb_ones[:], 1.0)

                # kv streaming pool allocated BEFORE io so its tiles
                # never alias io's SBUF (avoids WAR stalls on QT's reads)
                kv_cm = tc.tile_pool(name="kvk", bufs=8)
                kv = kv_cm.__enter__()
                kvv_cm = tc.tile_pool(name="kvv", bufs=4)
                kvv = kvv_cm.__enter__()

                # ---- projection phase ----
                with (
                    tc.tile_pool(name="io", bufs=1) as io,
                    tc.tile_pool(name="pp", bufs=4, space="PSUM") as pp,
                    tc.tile_pool(name="stage", bufs=4) as stage,
                ):
                    sb_xkv = io.tile([128, NCH * QPC], BF16, tag="xkv")
                    sb_wk = io.tile([128, NCH * D], BF16, tag="wk")
                    sb_wv = io.tile([128, NCH * D], BF16, tag="wv")
                    # consolidated input loads (one strided DMA each), K-h0
                    # critical path (wk + xkv-h0) first
                    def load_chunked(dst, src, cols):
                        nc.sync.dma_start(
                            dst.rearrange("p (c k) -> p c k", c=NCH)[:, :, 0:cols],
                            src.rearrange("(c p) k -> p c k", p=128),
                        )

                    # sync-queue FIFO order doubles as DMA priority
                    load_chunked(sb_xkv, xt_kv[:, 0:512], 512)
                    nc.sync.dma_start(
                        sb_wk.rearrange("p (c k) -> p c k", c=NCH)[:, :, 0:512],
                        wkT[:, 0:512].rearrange("(c p) k -> p c k", p=128),
                    )
                    nc.sync.dma_start(
                        sb_wk.rearrange("p (c k) -> p c k", c=NCH)[:, :, 512:1024],
                        wkT[:, 512:1024].rearrange("(c p) k -> p c k", p=128),
                    )
                    nc.sync.dma_start(
                        sb_mask.rearrange("k (t q) -> k t q", t=8),
                        masks.rearrange("t k q -> k t q"),
                    )
                    load_chunked(sb_wv, wvT, D)
                    nc.sync.dma_start(
                        sb_xkv.rearrange("p (c k) -> p c k", c=NCH)[:, :, 512:1024],
                        xt_kv[:, 512:1024].rearrange("(c p) k -> p c k", p=128),
                    )

                    def proj_group(lhs_sb, lhs_off, rhs_sb, rhs_off):
                        """8-chunk contraction matmul into a fresh PSUM tile.

                        All projection SBUF tiles share the layout
                        [128, 8*1024]: in-dim chunk c at cols [c*1024, ...).
                        """
                        ps = pp.tile([128, 512], F32, tag="pp", name="ps")
                        for c in range(NCH):
                            nc.tensor.matmul(
                                ps[:],
                                lhs_sb[:, c * 1024 + lhs_off : c * 1024 + lhs_off + 128],
                                rhs_sb[:, c * 1024 + rhs_off : c * 1024 + rhs_off + 512],
                                start=(c == 0),
                                stop=(c == NCH - 1),
                            )
                        return ps

                    def proj_group2(lhs_sb, lhs_off, rhs_sb, rhs_off0, rhs_off1):
                        """Two 512-wide outputs sharing the stationary operand
                        (back-to-back matmuls reuse the loaded weights)."""
                        ps0 = pp.tile([128, 512], F32, tag="pp", name="ps0")
                        ps1 = pp.tile([128, 512], F32, tag="pp", name="ps1")
                        for c in range(NCH):
                            lhs = lhs_sb[
                                :, c * 1024 + lhs_off : c * 1024 + lhs_off + 128
                            ]
                            nc.tensor.matmul(
                                ps0[:],
                                lhs,
                                rhs_sb[:, c * 1024 + rhs_off0 : c * 1024 + rhs_off0 + 512],
                                start=(c == 0),
                                stop=(c == NCH - 1),
                            )
                            nc.tensor.matmul(
                                ps1[:],
                                lhs,
                                rhs_sb[:, c * 1024 + rhs_off1 : c * 1024 + rhs_off1 + 512],
                                start=(c == 0),
                                stop=(c == NCH - 1),
                            )
                        return ps0, ps1


                    # K^T key-half h: rows = out-dim chunks oc, cols keys
                    # [512h, 512h+512); then V key-half h: key chunks kc.
                    for h in range(2):
                        for oc in range(NCH):
                            ps = proj_group(sb_wk, oc * 128, sb_xkv, h * 512)
                            stg = stage.tile([128, 512], BF16, tag="stg", name="stg")
                            nc.any.tensor_copy(stg[:], ps[:])
                            nc.sync.dma_start(
                                ag_k[h][oc * 128 : (oc + 1) * 128, :], stg[:]
                            )
                        nc.gpsimd.collective_compute(
                            "AllGather",
                            mybir.AluOpType.bypass,
                            replica_groups=rg,
                            ins=[ag_k[h].opt()],
                            outs=[g_k[h].opt()],
                        )
                        if h == 0:
                            load_chunked(sb_qt, xt_q, QPC)
                        for kc4 in range(4):
                            kc = h * 4 + kc4
                            ps0, ps1 = proj_group2(sb_xkv, kc * 128, sb_wv, 0, 512)
                            for dh, ps in ((0, ps0), (1, ps1)):
                                stg = stage.tile(
                                    [128, 512], BF16, tag="stg", name="stg"
                                )
                                nc.any.tensor_copy(stg[:], ps[:])
                                nc.sync.dma_start(
                                    ag_v[h][
                                        kc4 * 128 : (kc4 + 1) * 128,
                                        dh * 512 : (dh + 1) * 512,
                                    ],
                                    stg[:],
                                )
                        nc.gpsimd.collective_compute(
                            "AllGather",
                            mybir.AluOpType.bypass,
                            replica_groups=rg,
                            ins=[ag_v[h].opt()],
                            outs=[g_v[h].opt()],
                        )

                # ---- attention: two passes over key-halves, each pass split
                # into an ST phase (needs only gathered K^T) and a PV phase
                # (needs gathered V) so collectives hide behind compute ----
                with (
                    tc.tile_pool(name="oacc", bufs=1) as oaccp,
                    tc.tile_pool(name="psst", bufs=2, space="PSUM") as psst,
                    tc.tile_pool(name="pso", bufs=2, space="PSUM") as pso,
                ):
                    o_acc = [
                        oaccp.tile([128, D + 1], F32, tag=f"oacc{j}", name=f"oacc{j}")
                        for j in range(NQT)
                    ]

                    def j_groups(Sb):
                        """Contiguous J-tile ranges covering J in [Sb, 8)."""
                        if Sb + 4 < NQT:
                            return [(Sb, Sb + 4), (Sb + 4, NQT)]
                        return [(Sb, NQT)]

                    for H in range(2):
                        with tc.tile_pool(name=f"ptp{H}", bufs=1) as ptp:
                            pts = {}
                            # -- ST block: S^T = K^T.T @ Q^T, exp, mask --
                            def st_block(Sb):
                                    kt_t = kv.tile(
                                        [128, NCH * 512], BF16, tag="kt", name="kt_t"
                                    )
                                    for cp in range(NCH):
                                        eng = nc.gpsimd if cp == 7 else nc.sync
                                        eng.dma_start(
                                            kt_t[:, cp * 512 : (cp + 1) * 512],
                                            g_k[H][Sb, 128 * cp : 128 * (cp + 1), :],
                                        )
                                    for kt4 in range(4):
                                        kt = H * 4 + kt4
                                        for (j0, j1) in j_groups(Sb):
                                            N = (j1 - j0) * 128
                                            # Diagonal group: queries below q0 are
                                            # fully masked for this key tile (for
                                            # every core: 128*kt > 8*q+7), so skip
                                            # their ST columns.  exp reads stale
                                            # PSUM there (finite) and the mask
                                            # multiply zeroes it.
                                            q0 = max(0, 16 * kt - 1) if j0 == Sb else 0
                                            st = psst.tile(
                                                [128, 512], F32, tag="st", name="st"
                                            )
                                            for c in range(NCH):
                                                nc.tensor.matmul(
                                                    st[:, q0:N],
                                                    kt_t[
                                                        :,
                                                        c * 512
                                                        + kt4 * 128 : c * 512
                                                        + kt4 * 128
                                                        + 128,
                                                    ],
                                                    sb_qt[
                                                        :,
                                                        c * QPC
                                                        + j0 * 128
                                                        + q0 : c * QPC
                                                        + j1 * 128,
                                                    ],
                                                    start=(c == 0),
                                                    stop=(c == NCH - 1),
                                                )
                                            pt = ptp.tile(
                                                [128, N],
                                                BF16,
                                                tag=f"pt{Sb}_{j0}_{kt4}",
                                                name=f"pt{Sb}_{j0}_{kt4}",
                                            )
                                            nc.scalar.activation(
                                                pt[:],
                                                st[:, 0:N],
                                                mybir.ActivationFunctionType.Exp,
                                                scale=float(SCALE),
                                            )
                                            if j0 == Sb:
                                                # first J-tile of the group is the
                                                # causal diagonal -> mask it
                                                nc.vector.tensor_mul(
                                                    pt[:, 0:128],
                                                    pt[:, 0:128],
                                                    sb_mask[:, kt * 128 : kt * 128 + 128],
                                                )
                                            pts[(Sb, j0, kt4)] = pt

                            # -- PV block: O += P^T.T @ V, denom via ones --
                            def pv_block(Sb):
                                    v_t = kvv.tile([128, 4 * D], BF16, tag="v", name="v_t")
                                    for cp in range(4):
                                        nc.sync.dma_start(
                                            v_t[:, cp * D : (cp + 1) * D],
                                            g_v[H][Sb, 128 * cp : 128 * (cp + 1), :],
                                        )
                                    for J in range(Sb, NQT):
                                        j0 = Sb if J < min(Sb + 4, NQT) else Sb + 4
                                        o_ps = pso.tile(
                                            [128, 1536], F32, tag="ops", name="o_ps"
                                        )
                                        for kt4 in range(4):
                                            pt = pts[(Sb, j0, kt4)]
                                            lhsT = pt[:, (J - j0) * 128 : (J - j0 + 1) * 128]
                                            nc.tensor.matmul(
                                                o_ps[:, 0:512],
                                                lhsT,
                                                v_t[:, kt4 * D : kt4 * D + 512],
                                                start=(kt4 == 0),
                                                stop=(kt4 == 3),
                                            )
                                            nc.tensor.matmul(
                                                o_ps[:, 512:1024],
                                                lhsT,
                                                v_t[:, kt4 * D + 512 : kt4 * D + 1024],
                                                start=(kt4 == 0),
                                                stop=(kt4 == 3),
                                            )
                                            nc.tensor.matmul(
                                                o_ps[:, 1024:1025],
                                                lhsT,
                                                sb_ones[:],
                                                start=(kt4 == 0),
                                                stop=(kt4 == 3),
                                            )

                                        if H == 0 and Sb == 0:
                                            nc.vector.tensor_copy(
                                                o_acc[J][:], o_ps[:, 0 : D + 1]
                                            )
                                        else:
                                            nc.vector.tensor_add(
                                                o_acc[J][:], o_acc[J][:], o_ps[:, 0 : D + 1]
                                            )

                                        if H == 1 and Sb == J:
                                            rs = fin.tile([128, 1], F32, tag="rs", name="rs")
                                            nc.vector.reciprocal(
                                                rs[:], o_acc[J][:, D : D + 1]
                                            )
                                            outt = fin.tile(
                                                [128, D], F32, tag="outt", name="outt"
                                            )
                                            nc.vector.tensor_scalar_mul(
                                                outt[:], o_acc[J][:, 0:D], rs[:]
                                            )
                                            nc.sync.dma_start(
                                                out[J * 128 : (J + 1) * 128, :], outt[:]
                                            )

                            if H == 0:
                                # interleave aligned with stream arrival: PE
                                # reaches pv(0) at ~K1+58us, V1 lands at
                                # ~K1+50us, and the sync-ring FIFO order
                                # (kt0-3, v0, kt4, v1, ...) matches the
                                # consumption order with slack at each step
                                for Sb in range(4):
                                    st_block(Sb)
                                pv_block(0)
                                st_block(4)
                                pv_block(1)
                                st_block(5)
                                pv_block(2)
                                st_block(6)
                                pv_block(3)
                                st_block(7)
                                for Sb in range(4, NC):
                                    pv_block(Sb)
                            else:
                                for Sb in range(NC):
                                    st_block(Sb)
                                for Sb in range(NC):
                                    pv_block(Sb)
                kvv_cm.__exit__(None, None, None)
                kv_cm.__exit__(None, None, None)

    nc.compile()
    _cache["nc"] = nc
    return nc


def _make_in_maps(inputs, w_query, w_key, w_value):
    bf = ml_dtypes.bfloat16
    xt = np.ascontiguousarray(inputs.T.astype(np.float32))  # [D, S]
    # Wq absorbed into the key path: scores = x_k^T (Wk^T Wq) x_q
    wkT = np.ascontiguousarray(
        w_key.T.astype(np.float32) @ w_query.astype(np.float32)
    ).astype(bf)
    wvT = np.ascontiguousarray(w_value.T).astype(bf)

    kt_off = np.arange(8)[:, None, None] * 128 + np.arange(128)[None, :, None]
    in_maps = []
    for i in range(NC):
        xkv = np.ascontiguousarray(xt[:, i * QPC : (i + 1) * QPC]).astype(bf)
        xq = np.ascontiguousarray(xt[:, i::NC]).astype(bf)
        q_off = np.arange(128)[None, None, :] * 8 + i
        m = (kt_off <= q_off).astype(np.float32).astype(bf)  # [8,128,128]
        in_maps.append(
            {
                "xt_kv": xkv,
                "xt_q": xq,
                "wkT": wkT,
                "wvT": wvT,
                "masks": np.ascontiguousarray(m),
            }
        )
    return in_maps


def run(inputs, w_query, w_key, w_value, trace=False):
    nc = _build()
    in_maps = _make_in_maps(inputs, w_query, w_key, w_value)
    res = bass_utils.run_bass_kernel_spmd(
        nc, in_maps, core_ids=list(range(NC)), trace=trace
    )
    full = np.empty((S, D), dtype=np.float32)
    for i in range(NC):
        full[i::NC] = res.results[i]["out"]
    return full, res


def kernel(inputs, w_query, w_key, w_value):
    inputs = np.asarray(inputs, dtype=np.float32)
    w_query = np.asarray(w_query, dtype=np.float32)
    w_key = np.asarray(w_key, dtype=np.float32)
    w_value = np.asarray(w_value, dtype=np.float32)
    full, _ = run(inputs, w_query, w_key, w_value, trace=False)
    return full



# revision 3
# speedup vs baseline: 1.0316x; 1.0316x over previous
"""Causal self-attention (SEQ=8192, D=1024) on 8 TRN2 NeuronCores.

Strategy (SPMD, one static graph on all 8 cores):
  - Sequence parallel over queries with stride-8 row interleaving:
    core i owns query rows {8j+i : j in [0,1024)}. This balances causal
    work exactly while keeping the instruction graph identical across
    cores (per-core differences are pure data: X^T slices + masks).
  - Core i computes K^T/V projections for the contiguous key shard
    [1024*i, 1024*(i+1)). K^T/V are shared via SIX chunked AllGathers
    (K-h0 in two 256-key pieces, V-h0 in two 256-key pieces, K-h1,
    V-h1), each triggered from an otherwise-empty GpSimd queue the
    moment its projection slice lands in DRAM, so the first gathered
    keys arrive ~as the projection phase drains.
  - Attention runs in S^T layout ([keys x queries]): S^T = K^T.T @ Q^T,
    so softmax(P)^T is directly the lhsT for P@V -- no transposes.
    Per key-half: ST for all shards first (piece-a then piece-b for
    h0), then PV in J-major order with a single PSUM accumulation
    group per (J, kt-piece) spanning all source shards -- one vector
    eviction per group instead of one per (shard, J).
    exp on ScalarE (scale fused), no max-subtraction (scores are
    N(0,1)-scaled), denominator via a ones-column matmul accumulated
    alongside O in PSUM.
  - All matmul operands bf16 (1 cyc/row on the PE), accumulation fp32.
  - DMA queues: sync = input loads -> ag writebacks -> gather reads ->
    output stores (FIFO order matches data-readiness order); vector =
    Q^T load; gpsimd = collective triggers only.
"""
import sys

sys.path.insert(0, "/opt/trn_rl_repo")

import numpy as np
import ml_dtypes

import concourse.bacc as bacc
import concourse.mybir as mybir
import concourse.tile as tile
from concourse import bass_utils

S, D, NC = 8192, 1024, 8
QPC = S // NC  # 1024 queries (and kv rows) per core
NCH = D // 128  # 8 chunks of the feature dim
NQT = QPC // 128  # 8 query tiles per core
SCALE = 1.0 / np.sqrt(D).astype(np.float32)  # 1/32
BF16 = mybir.dt.bfloat16
F32 = mybir.dt.float32

_cache = {}


def _j_groups(Sb):
    """Contiguous J-tile ranges covering J in [Sb, 8)."""
    if Sb + 4 < NQT:
        return [(Sb, Sb + 4), (Sb + 4, NQT)]
    return [(Sb, NQT)]


def _build():
    if "nc" in _cache:
        return _cache["nc"]
    nc = bacc.Bacc("TRN2", target_bir_lowering=False, debug=False, num_devices=NC)

    xt_kv = nc.dram_tensor("xt_kv", [D, QPC], BF16, kind="ExternalInput")
    xt_q = nc.dram_tensor("xt_q", [D, QPC], BF16, kind="ExternalInput")
    wkT = nc.dram_tensor("wkT", [D, D], BF16, kind="ExternalInput")
    wvT = nc.dram_tensor("wvT", [D, D], BF16, kind="ExternalInput")
    masks = nc.dram_tensor("masks", [8, 128, 128], BF16, kind="ExternalInput")
    out = nc.dram_tensor("out", [QPC, D], F32, kind="ExternalOutput")

    rg = [list(range(NC))]

    def all_gather(src, dst):
        nc.gpsimd.collective_compute(
            "AllGather",
            mybir.AluOpType.bypass,
            replica_groups=rg,
            ins=[src.opt()],
            outs=[dst.opt()],
        )

    with tile.TileContext(nc) as tc:
        with tc.tile_pool(name="dram", bufs=1, space="DRAM") as dram:
            # AllGather bounce buffers. K-h0/V-h0 split into 256-key
            # pieces so the first gathered keys land early.
            ag_ka = dram.tile([D, 256], BF16, name="agka")
            ag_kb = dram.tile([D, 256], BF16, name="agkb")
            ag_k1 = dram.tile([D, 512], BF16, name="agk1")
            ag_va = dram.tile([256, D], BF16, name="agva")
            ag_vb = dram.tile([256, D], BF16, name="agvb")
            ag_v1 = dram.tile([512, D], BF16, name="agv1")
            g_ka = dram.tile([NC, D, 256], BF16, addr_space="Shared", name="gka")
            g_kb = dram.tile([NC, D, 256], BF16, addr_space="Shared", name="gkb")
            g_k1 = dram.tile([NC, D, 512], BF16, addr_space="Shared", name="gk1")
            g_va = dram.tile([NC, 256, D], BF16, addr_space="Shared", name="gva")
            g_vb = dram.tile([NC, 256, D], BF16, addr_space="Shared", name="gvb")
            g_v1 = dram.tile([NC, 512, D], BF16, addr_space="Shared", name="gv1")

            with (
                tc.tile_pool(name="persist", bufs=1) as persist,
                tc.tile_pool(name="fin", bufs=2) as fin,
            ):
                sb_qt = persist.tile([128, NCH * QPC], BF16, tag="qt")
                sb_mask = persist.tile([128, 8 * 128], BF16, tag="msk")
                sb_ones = persist.tile([128, 1], BF16, tag="ones")
                nc.vector.memset(sb_ones[:], 1.0)
                o_acc = [
                    persist.tile([128, D + 1], F32, tag=f"oacc{j}", name=f"oacc{j}")
                    for j in range(NQT)
                ]

                # Q^T load on the scalar queue (its first activation comes
                # long after this lands; keeps sync free for K-path loads).
                nc.scalar.dma_start(
                    sb_qt.rearrange("p (c k) -> p c k", c=NCH),
                    xt_q.rearrange("(c p) k -> p c k", p=128),
                )

                # ---- projection phase ----
                with (
                    tc.tile_pool(name="io", bufs=1) as io,
                    tc.tile_pool(name="pp", bufs=4, space="PSUM") as pp,
                    tc.tile_pool(name="stage", bufs=4) as stage,
                ):
                    sb_xkv = io.tile([128, NCH * QPC], BF16, tag="xkv")
                    sb_wk = io.tile([128, NCH * D], BF16, tag="wk")
                    sb_wv = io.tile([128, NCH * D], BF16, tag="wv")
                    # sync-queue FIFO order doubles as DMA priority:
                    # K-h0 critical path (wk + xkv-h0) first
                    nc.sync.dma_start(
                        sb_xkv.rearrange("p (c k) -> p c k", c=NCH)[:, :, 0:512],
                        xt_kv[:, 0:512].rearrange("(c p) k -> p c k", p=128),
                    )
                    nc.sync.dma_start(
                        sb_wk.rearrange("p (c k) -> p c k", c=NCH)[:, :, 0:512],
                        wkT[:, 0:512].rearrange("(c p) k -> p c k", p=128),
                    )
                    nc.sync.dma_start(
                        sb_wk.rearrange("p (c k) -> p c k", c=NCH)[:, :, 512:1024],
                        wkT[:, 512:1024].rearrange("(c p) k -> p c k", p=128),
                    )
                    nc.sync.dma_start(
                        sb_mask.rearrange("k (t q) -> k t q", t=8),
                        masks.rearrange("t k q -> k t q"),
                    )
                    nc.sync.dma_start(
                        sb_wv.rearrange("p (c k) -> p c k", c=NCH),
                        wvT.rearrange("(c p) k -> p c k", p=128),
                    )
                    nc.sync.dma_start(
                        sb_xkv.rearrange("p (c k) -> p c k", c=NCH)[:, :, 512:1024],
                        xt_kv[:, 512:1024].rearrange("(c p) k -> p c k", p=128),
                    )

                    def proj_group(lhs_sb, lhs_off, rhs_sb, rhs_off):
                        """8-chunk contraction matmul into a fresh PSUM tile.

                        All projection SBUF tiles share the layout
                        [128, 8*1024]: in-dim chunk c at cols [c*1024, ...).
                        """
                        ps = pp.tile([128, 512], F32, tag="pp", name="ps")
                        for c in range(NCH):
                            nc.tensor.matmul(
                                ps[:],
                                lhs_sb[:, c * 1024 + lhs_off : c * 1024 + lhs_off + 128],
                                rhs_sb[:, c * 1024 + rhs_off : c * 1024 + rhs_off + 512],
                                start=(c == 0),
                                stop=(c == NCH - 1),
                            )
                        return ps

                    def proj_group2(lhs_sb, lhs_off, rhs_sb, rhs_off0, rhs_off1):
                        """Two 512-wide outputs sharing the stationary operand
                        (back-to-back matmuls reuse the loaded weights)."""
                        ps0 = pp.tile([128, 512], F32, tag="pp", name="ps0")
                        ps1 = pp.tile([128, 512], F32, tag="pp", name="ps1")
                        for c in range(NCH):
                            lhs = lhs_sb[
                                :, c * 1024 + lhs_off : c * 1024 + lhs_off + 128
                            ]
                            nc.tensor.matmul(
                                ps0[:],
                                lhs,
                                rhs_sb[:, c * 1024 + rhs_off0 : c * 1024 + rhs_off0 + 512],
                                start=(c == 0),
                                stop=(c == NCH - 1),
                            )
                            nc.tensor.matmul(
                                ps1[:],
                                lhs,
                                rhs_sb[:, c * 1024 + rhs_off1 : c * 1024 + rhs_off1 + 512],
                                start=(c == 0),
                                stop=(c == NCH - 1),
                            )
                        return ps0, ps1

                    def evict(ps, cols, dst_ap):
                        stg = stage.tile([128, cols], BF16, tag="stg", name="stg")
                        nc.any.tensor_copy(stg[:], ps)
                        nc.sync.dma_start(dst_ap, stg[:])

                    # K-h0: rows = out-dim chunks oc, split into key pieces
                    for oc in range(NCH):
                        ps = proj_group(sb_wk, oc * 128, sb_xkv, 0)
                        evict(ps[:, 0:256], 256, ag_ka[oc * 128 : (oc + 1) * 128, :])
                        evict(ps[:, 256:512], 256, ag_kb[oc * 128 : (oc + 1) * 128, :])
                    all_gather(ag_ka, g_ka)
                    all_gather(ag_kb, g_kb)

                    # V-h0: key chunks kc4; rows 0-1 -> piece a, 2-3 -> piece b
                    for kc4 in range(4):
                        ps0, ps1 = proj_group2(sb_xkv, kc4 * 128, sb_wv, 0, 512)
                        dst = ag_va if kc4 < 2 else ag_vb
                        row = (kc4 % 2) * 128
                        for dh, ps in ((0, ps0), (1, ps1)):
                            evict(
                                ps[:],
                                512,
                                dst[row : row + 128, dh * 512 : (dh + 1) * 512],
                            )
                        if kc4 == 1:
                            all_gather(ag_va, g_va)
                    all_gather(ag_vb, g_vb)

                    # K-h1
                    for oc in range(NCH):
                        ps = proj_group(sb_wk, oc * 128, sb_xkv, 512)
                        evict(ps[:], 512, ag_k1[oc * 128 : (oc + 1) * 128, :])
                    all_gather(ag_k1, g_k1)

                    # V-h1
                    for kc4 in range(4):
                        ps0, ps1 = proj_group2(sb_xkv, 512 + kc4 * 128, sb_wv, 0, 512)
                        for dh, ps in ((0, ps0), (1, ps1)):
                            evict(
                                ps[:],
                                512,
                                ag_v1[
                                    kc4 * 128 : (kc4 + 1) * 128,
                                    dh * 512 : (dh + 1) * 512,
                                ],
                            )
                    all_gather(ag_v1, g_v1)

                # ---- attention: per key-half, ST for all shards then PV
                # in J-major order (single PSUM group per (J, kt-piece)
                # spanning all shards -> one eviction per group) ----

                def st_groups(pts, Sb, kt, lhsT_of):
                    """Score-transpose groups for (shard Sb, key tile kt).

                    lhsT_of(c) gives the [128, 128] stationary K^T chunk.
                    Fills pts[(Sb, j0, kt)] with exp'd (masked) P tiles.
                    """
                    for (j0, j1) in _j_groups(Sb):
                        N = (j1 - j0) * 128
                        # Diagonal group: queries below q0 are fully masked
                        # for this key tile (for every core: 128*kt > 8*q+7),
                        # so skip their ST columns. exp reads stale PSUM
                        # there (finite) and the mask multiply zeroes it.
                        q0 = max(0, 16 * kt - 1) if j0 == Sb else 0
                        st = psst.tile([128, 512], F32, tag="st", name="st")
                        for c in range(NCH):
                            nc.tensor.matmul(
                                st[:, q0:N],
                                lhsT_of(c),
                                sb_qt[
                                    :,
                                    c * QPC + j0 * 128 + q0 : c * QPC + j1 * 128,
                                ],
                                start=(c == 0),
                                stop=(c == NCH - 1),
                            )
                        pt = ptp.tile(
                            [128, N],
                            BF16,
                            tag=f"pt{Sb}_{j0}_{kt}",
                            name=f"pt{Sb}_{j0}_{kt}",
                        )
                        nc.scalar.activation(
                            pt[:],
                            st[:, 0:N],
                            mybir.ActivationFunctionType.Exp,
                            scale=float(SCALE),
                        )
                        if j0 == Sb:
                            nc.vector.tensor_mul(
                                pt[:, 0:128],
                                pt[:, 0:128],
                                sb_mask[:, kt * 128 : kt * 128 + 128],
                            )
                        pts[(Sb, j0, kt)] = pt

                def pv_pass(pts, v_of, kts, first, last):
                    """J-major PV: one PSUM group per J over all shards and
                    the key tiles `kts`; evict into o_acc (copy if `first`),
                    finalize + store if `last`."""
                    for J in range(NQT):
                        o_ps = pso.tile([128, 1536], F32, tag="ops", name="o_ps")
                        chunks = [
                            (Sb, kt) for Sb in range(J + 1) for kt in kts
                        ]
                        for idx, (Sb, kt) in enumerate(chunks):
                            j0 = Sb if J < min(Sb + 4, NQT) else Sb + 4
                            pt = pts[(Sb, j0, kt)]
                            lhsT = pt[:, (J - j0) * 128 : (J - j0 + 1) * 128]
                            rhs = v_of(Sb, kt)
                            st_f = idx == 0
                            sp_f = idx == len(chunks) - 1
                            nc.tensor.matmul(
                                o_ps[:, 0:512], lhsT, rhs[0],
                                start=st_f, stop=sp_f,
                            )
                            nc.tensor.matmul(
                                o_ps[:, 512:1024], lhsT, rhs[1],
                                start=st_f, stop=sp_f,
                            )
                            nc.tensor.matmul(
                                o_ps[:, 1024:1025], lhsT, sb_ones[:],
                                start=st_f, stop=sp_f,
                            )
                        if first:
                            nc.vector.tensor_copy(o_acc[J][:], o_ps[:, 0 : D + 1])
                        else:
                            nc.vector.tensor_add(
                                o_acc[J][:], o_acc[J][:], o_ps[:, 0 : D + 1]
                            )
                        if last:
                            rs = fin.tile([128, 1], F32, tag="rs", name="rs")
                            nc.vector.reciprocal(rs[:], o_acc[J][:, D : D + 1])
                            outt = fin.tile([128, D], F32, tag="outt", name="outt")
                            nc.vector.tensor_scalar_mul(
                                outt[:], o_acc[J][:, 0:D], rs[:]
                            )
                            nc.sync.dma_start(
                                out[J * 128 : (J + 1) * 128, :], outt[:]
                            )

                # ---- key-half 0 ----
                with tc.tile_pool(name="pt0", bufs=1) as ptp:
                    pts = {}
                    with (
                        tc.tile_pool(name="k0p", bufs=16) as kp,
                        tc.tile_pool(name="psst", bufs=2, space="PSUM") as psst,
                    ):
                        for piece, gk in ((0, g_ka), (1, g_kb)):
                            for Sb in range(NC):
                                kt_t = kp.tile(
                                    [128, NCH * 256],
                                    BF16,
                                    tag="kt",
                                    name=f"kt{piece}_{Sb}",
                                )
                                nc.sync.dma_start(
                                    kt_t.rearrange("p (c k) -> p c k", c=NCH),
                                    gk[Sb].rearrange("(c p) k -> p c k", p=128),
                                )
                                for k2 in range(2):
                                    kt = piece * 2 + k2
                                    st_groups(
                                        pts,
                                        Sb,
                                        kt,
                                        lambda c, kt_t=kt_t, k2=k2: kt_t[
                                            :,
                                            c * 256 + k2 * 128 : c * 256
                                            + k2 * 128
                                            + 128,
                                        ],
                                    )
                    with (
                        tc.tile_pool(name="v0p", bufs=16) as vp,
                        tc.tile_pool(name="pso", bufs=2, space="PSUM") as pso,
                    ):
                        v_ts = {}
                        for piece, gv in ((0, g_va), (1, g_vb)):
                            for Sb in range(NC):
                                v_t = vp.tile(
                                    [128, 2 * D],
                                    BF16,
                                    tag="v",
                                    name=f"v{piece}_{Sb}",
                                )
                                nc.sync.dma_start(
                                    v_t.rearrange("p (c d) -> p c d", c=2),
                                    gv[Sb].rearrange("(c p) d -> p c d", p=128),
                                )
                                v_ts[(piece, Sb)] = v_t

                        def v_of0(piece):
                            def f(Sb, kt):
                                k2 = kt - piece * 2
                                v_t = v_ts[(piece, Sb)]
                                return (
                                    v_t[:, k2 * D : k2 * D + 512],
                                    v_t[:, k2 * D + 512 : k2 * D + 1024],
                                )

                            return f

                        pv_pass(pts, v_of0(0), (0, 1), first=True, last=False)
                        pv_pass(pts, v_of0(1), (2, 3), first=False, last=False)

                # ---- key-half 1 ----
                with tc.tile_pool(name="pt1", bufs=1) as ptp:
                    pts = {}
                    with (
                        tc.tile_pool(name="k1p", bufs=4) as kp,
                        tc.tile_pool(name="psst", bufs=2, space="PSUM") as psst,
                    ):
                        for Sb in range(NC):
                            kt_t = kp.tile(
                                [128, NCH * 512], BF16, tag="kt", name=f"kt1_{Sb}"
                            )
                            nc.sync.dma_start(
                                kt_t.rearrange("p (c k) -> p c k", c=NCH),
                                g_k1[Sb].rearrange("(c p) k -> p c k", p=128),
                            )
                            for k4 in range(4):
                                kt = 4 + k4
                                st_groups(
                                    pts,
                                    Sb,
                                    kt,
                                    lambda c, kt_t=kt_t, k4=k4: kt_t[
                                        :,
                                        c * 512 + k4 * 128 : c * 512 + k4 * 128 + 128,
                                    ],
                                )
                    with (
                        tc.tile_pool(name="v1p", bufs=8) as vp,
                        tc.tile_pool(name="pso", bufs=2, space="PSUM") as pso,
                    ):
                        v_ts = {}
                        for Sb in range(NC):
                            v_t = vp.tile(
                                [128, 4 * D], BF16, tag="v", name=f"v1_{Sb}"
                            )
                            nc.sync.dma_start(
                                v_t.rearrange("p (c d) -> p c d", c=4),
                                g_v1[Sb].rearrange("(c p) d -> p c d", p=128),
                            )
                            v_ts[Sb] = v_t

                        def v_of1(Sb, kt):
                            k4 = kt - 4
                            v_t = v_ts[Sb]
                            return (
                                v_t[:, k4 * D : k4 * D + 512],
                                v_t[:, k4 * D + 512 : k4 * D + 1024],
                            )

                        pv_pass(pts, v_of1, (4, 5, 6, 7), first=False, last=True)

    nc.compile()
    _cache["nc"] = nc
    return nc


def _make_in_maps(inputs, w_query, w_key, w_value):
    bf = ml_dtypes.bfloat16
    xt = np.ascontiguousarray(inputs.T.astype(np.float32))  # [D, S]
    # Wq absorbed into the key path: scores = x_k^T (Wk^T Wq) x_q
    wkT = np.ascontiguousarray(
        w_key.T.astype(np.float32) @ w_query.astype(np.float32)
    ).astype(bf)
    wvT = np.ascontiguousarray(w_value.T).astype(bf)

    kt_off = np.arange(8)[:, None, None] * 128 + np.arange(128)[None, :, None]
    in_maps = []
    for i in range(NC):
        xkv = np.ascontiguousarray(xt[:, i * QPC : (i + 1) * QPC]).astype(bf)
        xq = np.ascontiguousarray(xt[:, i::NC]).astype(bf)
        q_off = np.arange(128)[None, None, :] * 8 + i
        m = (kt_off <= q_off).astype(np.float32).astype(bf)  # [8,128,128]
        in_maps.append(
            {
                "xt_kv": xkv,
                "xt_q": xq,
                "wkT": wkT,
                "wvT": wvT,
                "masks": np.ascontiguousarray(m),
            }
        )
    return in_maps


def run(inputs, w_query, w_key, w_value, trace=False):
    nc = _build()
    in_maps = _make_in_maps(inputs, w_query, w_key, w_value)
    res = bass_utils.run_bass_kernel_spmd(
        nc, in_maps, core_ids=list(range(NC)), trace=trace
    )
    full = np.empty((S, D), dtype=np.float32)
    for i in range(NC):
        full[i::NC] = res.results[i]["out"]
    return full, res


def kernel(inputs, w_query, w_key, w_value):
    inputs = np.asarray(inputs, dtype=np.float32)
    w_query = np.asarray(w_query, dtype=np.float32)
    w_key = np.asarray(w_key, dtype=np.float32)
    w_value = np.asarray(w_value, dtype=np.float32)
    full, _ = run(inputs, w_query, w_key, w_value, trace=False)
    return full


# revision 6
# speedup vs baseline: 1.0674x; 1.0347x over previous
"""Causal self-attention (SEQ=8192, D=1024) on 8 TRN2 NeuronCores.

Strategy (SPMD, one static graph on all 8 cores):
  - Sequence parallel over queries with stride-8 row interleaving:
    core i owns query rows {8j+i : j in [0,1024)}. This balances causal
    work exactly while keeping the instruction graph identical across
    cores (per-core differences are pure data: X^T slices + masks).
  - Core i computes K^T/V projections for the contiguous key shard
    [1024*i, 1024*(i+1)). K^T/V are shared via SIX chunked AllGathers
    (K-h0 and V-h0 in two 256-key pieces each, K-h1, V-h1), each
    triggered from an otherwise-idle GpSimd queue the moment its
    projection slice lands in DRAM; the first gathered keys arrive
    roughly as the projection phase drains.
  - Attention runs in S^T layout ([keys x queries]): S^T = K^T.T @ Q^T,
    so softmax(P)^T is directly the lhsT for P@V -- no transposes.
    Per key-half: ST for all shards first, then PV in J-major order
    with one PSUM accumulation group per (J, kt-piece) spanning all
    shards -- one vector eviction per group.  exp on ScalarE (scale
    fused), no max-subtraction (scores are N(0,1)-scaled), denominator
    via a ones-column matmul accumulated alongside O in PSUM.
  - All matmul operands bf16 (1 cyc/row on the PE), accumulation fp32.
  - Queues: sync = xkv/mask/wv loads -> K gather reads -> out stores;
    scalar = wk/qt loads -> ag writebacks; gpsimd = collective
    triggers -> V gather reads.  FIFO order matches readiness order.
  - Pool lifetimes are arranged so every gather read lands in SBUF
    whose previous readers finished long before the gather completes
    (no WAR stalls): k1p reuses k0p's space (readers done at ST-H0
    end), v1p reuses v0p's space, P tiles share tags across halves.
"""
import sys

sys.path.insert(0, "/opt/trn_rl_repo")

import numpy as np
import ml_dtypes

import concourse.bacc as bacc
import concourse.mybir as mybir
import concourse.tile as tile
from concourse import bass_utils

S, D, NC = 8192, 1024, 8
QPC = S // NC  # 1024 queries (and kv rows) per core
NCH = D // 128  # 8 chunks of the feature dim
NQT = QPC // 128  # 8 query tiles per core
SCALE = 1.0 / np.sqrt(D).astype(np.float32)  # 1/32
BF16 = mybir.dt.bfloat16
F32 = mybir.dt.float32

_cache = {}


def _j_groups(Sb):
    """Contiguous J-tile ranges covering J in [Sb, 8)."""
    if Sb + 4 < NQT:
        return [(Sb, Sb + 4), (Sb + 4, NQT)]
    return [(Sb, NQT)]


def _build():
    if "nc" in _cache:
        return _cache["nc"]
    nc = bacc.Bacc("TRN2", target_bir_lowering=False, debug=False, num_devices=NC)

    xt_kv = nc.dram_tensor("xt_kv", [D, QPC], BF16, kind="ExternalInput")
    xt_q = nc.dram_tensor("xt_q", [D, QPC], BF16, kind="ExternalInput")
    wkT = nc.dram_tensor("wkT", [D, D], BF16, kind="ExternalInput")
    wvT = nc.dram_tensor("wvT", [D, D], BF16, kind="ExternalInput")
    masks = nc.dram_tensor("masks", [8, 128, 128], BF16, kind="ExternalInput")
    out = nc.dram_tensor("out", [QPC, D], F32, kind="ExternalOutput")

    rg = [list(range(NC))]

    def all_gather(src, dst):
        nc.gpsimd.collective_compute(
            "AllGather",
            mybir.AluOpType.bypass,
            replica_groups=rg,
            ins=[src.opt()],
            outs=[dst.opt()],
        )

    with tile.TileContext(nc) as tc:
        with tc.tile_pool(name="dram", bufs=1, space="DRAM") as dram:
            ag_ka = dram.tile([D, 256], BF16, name="agka")
            ag_kb = dram.tile([D, 256], BF16, name="agkb")
            ag_k1 = dram.tile([D, 512], BF16, name="agk1")
            ag_va = dram.tile([256, D], BF16, name="agva")
            ag_vb = dram.tile([256, D], BF16, name="agvb")
            ag_v1 = dram.tile([512, D], BF16, name="agv1")
            g_ka = dram.tile([NC, D, 256], BF16, addr_space="Shared", name="gka")
            g_kb = dram.tile([NC, D, 256], BF16, addr_space="Shared", name="gkb")
            g_k1 = dram.tile([NC, D, 512], BF16, addr_space="Shared", name="gk1")
            g_va = dram.tile([NC, 256, D], BF16, addr_space="Shared", name="gva")
            g_vb = dram.tile([NC, 256, D], BF16, addr_space="Shared", name="gvb")
            g_v1 = dram.tile([NC, 512, D], BF16, addr_space="Shared", name="gv1")

            with (
                tc.tile_pool(name="persist", bufs=1) as persist,
                tc.tile_pool(name="fin", bufs=2) as fin,
                tc.tile_pool(name="ptp", bufs=1) as ptp,
            ):
                sb_qt = persist.tile([128, NCH * QPC], BF16, tag="qt")
                sb_mask = persist.tile([128, 8 * 128], BF16, tag="msk")
                sb_ones = persist.tile([128, 1], BF16, tag="ones")
                nc.vector.memset(sb_ones[:], 1.0)
                o_acc = [
                    persist.tile([128, D + 1], F32, tag=f"oacc{j}", name=f"oacc{j}")
                    for j in range(NQT)
                ]

                # ---- projection phase ----
                with (
                    tc.tile_pool(name="io", bufs=1) as io,
                    tc.tile_pool(name="pp", bufs=4, space="PSUM") as pp,
                    tc.tile_pool(name="stage", bufs=8) as stage,
                ):
                    sb_xkv = io.tile([128, NCH * QPC], BF16, tag="xkv")
                    sb_wk = io.tile([128, NCH * D], BF16, tag="wk")
                    sb_wv = io.tile([128, NCH * D], BF16, tag="wv")
                    nc.sync.dma_start(
                        sb_xkv.rearrange("p (c k) -> p c k", c=NCH)[:, :, 0:512],
                        xt_kv[:, 0:512].rearrange("(c p) k -> p c k", p=128),
                    )
                    nc.scalar.dma_start(
                        sb_wk.rearrange("p (c k) -> p c k", c=NCH)[:, :, 0:512],
                        wkT[:, 0:512].rearrange("(c p) k -> p c k", p=128),
                    )
                    nc.scalar.dma_start(
                        sb_wk.rearrange("p (c k) -> p c k", c=NCH)[:, :, 512:1024],
                        wkT[:, 512:1024].rearrange("(c p) k -> p c k", p=128),
                    )
                    nc.scalar.dma_start(
                        sb_qt.rearrange("p (c k) -> p c k", c=NCH),
                        xt_q.rearrange("(c p) k -> p c k", p=128),
                    )
                    nc.sync.dma_start(
                        sb_mask.rearrange("k (t q) -> k t q", t=8),
                        masks.rearrange("t k q -> k t q"),
                    )
                    nc.sync.dma_start(
                        sb_wv.rearrange("p (c k) -> p c k", c=NCH),
                        wvT.rearrange("(c p) k -> p c k", p=128),
                    )
                    nc.sync.dma_start(
                        sb_xkv.rearrange("p (c k) -> p c k", c=NCH)[:, :, 512:1024],
                        xt_kv[:, 512:1024].rearrange("(c p) k -> p c k", p=128),
                    )

                    def proj_group(lhs_sb, lhs_off, rhs_sb, rhs_off):
                        """8-chunk contraction matmul into a fresh PSUM tile.

                        All projection SBUF tiles share the layout
                        [128, 8*1024]: in-dim chunk c at cols [c*1024, ...).
                        """
                        ps = pp.tile([128, 512], F32, tag="pp", name="ps")
                        for c in range(NCH):
                            nc.tensor.matmul(
                                ps[:],
                                lhs_sb[:, c * 1024 + lhs_off : c * 1024 + lhs_off + 128],
                                rhs_sb[:, c * 1024 + rhs_off : c * 1024 + rhs_off + 512],
                                start=(c == 0),
                                stop=(c == NCH - 1),
                            )
                        return ps

                    def proj_group2(lhs_sb, lhs_off, rhs_sb, rhs_off0, rhs_off1):
                        """Two 512-wide outputs sharing the stationary operand
                        (back-to-back matmuls reuse the loaded weights)."""
                        ps0 = pp.tile([128, 512], F32, tag="pp", name="ps0")
                        ps1 = pp.tile([128, 512], F32, tag="pp", name="ps1")
                        for c in range(NCH):
                            lhs = lhs_sb[
                                :, c * 1024 + lhs_off : c * 1024 + lhs_off + 128
                            ]
                            nc.tensor.matmul(
                                ps0[:],
                                lhs,
                                rhs_sb[:, c * 1024 + rhs_off0 : c * 1024 + rhs_off0 + 512],
                                start=(c == 0),
                                stop=(c == NCH - 1),
                            )
                            nc.tensor.matmul(
                                ps1[:],
                                lhs,
                                rhs_sb[:, c * 1024 + rhs_off1 : c * 1024 + rhs_off1 + 512],
                                start=(c == 0),
                                stop=(c == NCH - 1),
                            )
                        return ps0, ps1

                    def evict(ps, cols, dst_ap):
                        stg = stage.tile([128, cols], BF16, tag="stg", name="stg")
                        nc.any.tensor_copy(stg[:], ps)
                        nc.scalar.dma_start(dst_ap, stg[:])

                    # K-h0: rows = out-dim chunks oc, split into key pieces
                    for oc in range(NCH):
                        ps = proj_group(sb_wk, oc * 128, sb_xkv, 0)
                        evict(ps[:, 0:256], 256, ag_ka[oc * 128 : (oc + 1) * 128, :])
                        evict(ps[:, 256:512], 256, ag_kb[oc * 128 : (oc + 1) * 128, :])
                    all_gather(ag_ka, g_ka)
                    all_gather(ag_kb, g_kb)

                    # V-h0: key chunks kc4; rows 0-1 -> piece a, 2-3 -> piece b
                    for kc4 in range(4):
                        ps0, ps1 = proj_group2(sb_xkv, kc4 * 128, sb_wv, 0, 512)
                        dst = ag_va if kc4 < 2 else ag_vb
                        row = (kc4 % 2) * 128
                        for dh, ps in ((0, ps0), (1, ps1)):
                            evict(
                                ps[:],
                                512,
                                dst[row : row + 128, dh * 512 : (dh + 1) * 512],
                            )
                        if kc4 == 1:
                            all_gather(ag_va, g_va)
                    all_gather(ag_vb, g_vb)

                    # K-h1
                    for oc in range(NCH):
                        ps = proj_group(sb_wk, oc * 128, sb_xkv, 512)
                        evict(ps[:], 512, ag_k1[oc * 128 : (oc + 1) * 128, :])
                    all_gather(ag_k1, g_k1)

                    # V-h1
                    for kc4 in range(4):
                        ps0, ps1 = proj_group2(sb_xkv, 512 + kc4 * 128, sb_wv, 0, 512)
                        for dh, ps in ((0, ps0), (1, ps1)):
                            evict(
                                ps[:],
                                512,
                                ag_v1[
                                    kc4 * 128 : (kc4 + 1) * 128,
                                    dh * 512 : (dh + 1) * 512,
                                ],
                            )
                    all_gather(ag_v1, g_v1)

                # ---- attention ----
                pts = {}

                def st_groups(psst, Sb, kt, lhsT_of):
                    """Score-transpose groups for (shard Sb, key tile kt).

                    lhsT_of(c) gives the [128, 128] stationary K^T chunk.
                    Fills pts[(Sb, j0, kt)] with exp'd (masked) P tiles.
                    P tiles share tags across key-halves (same shapes), so
                    half-1 reuses half-0's buffers once PV-h0 is done.
                    """
                    for (j0, j1) in _j_groups(Sb):
                        N = (j1 - j0) * 128
                        # Diagonal group: queries below q0 are fully masked
                        # for this key tile (for every core: 128*kt > 8*q+7),
                        # so skip their ST columns; the pt prefix is zeroed
                        # explicitly (PSUM there is never written).
                        q0 = max(0, 16 * kt - 1) if j0 == Sb else 0
                        st = psst.tile([128, 512], F32, tag="st", name="st")
                        for c in range(NCH):
                            nc.tensor.matmul(
                                st[:, q0:N],
                                lhsT_of(c),
                                sb_qt[
                                    :,
                                    c * QPC + j0 * 128 + q0 : c * QPC + j1 * 128,
                                ],
                                start=(c == 0),
                                stop=(c == NCH - 1),
                            )
                        pt = ptp.tile(
                            [128, N],
                            BF16,
                            tag=f"pt{Sb}_{j0}_{kt % 4}",
                            name=f"pt{Sb}_{j0}_{kt}",
                        )
                        if q0 > 0:
                            nc.vector.memset(pt[:, 0:q0], 0.0)
                        nc.scalar.activation(
                            pt[:, q0:N],
                            st[:, q0:N],
                            mybir.ActivationFunctionType.Exp,
                            scale=float(SCALE),
                        )
                        if j0 == Sb:
                            nc.vector.tensor_mul(
                                pt[:, 0:128],
                                pt[:, 0:128],
                                sb_mask[:, kt * 128 : kt * 128 + 128],
                            )
                        pts[(Sb, j0, kt)] = pt

                def pv_pass(pso, v_of, kts, first, last):
                    """J-major PV: one PSUM group per J over all shards and
                    the key tiles `kts`; evict into o_acc (copy if `first`),
                    finalize + store if `last`."""
                    for J in range(NQT):
                        o_ps = pso.tile([128, 1536], F32, tag="ops", name="o_ps")
                        chunks = [(Sb, kt) for Sb in range(J + 1) for kt in kts]
                        for idx, (Sb, kt) in enumerate(chunks):
                            j0 = Sb if J < min(Sb + 4, NQT) else Sb + 4
                            pt = pts[(Sb, j0, kt)]
                            lhsT = pt[:, (J - j0) * 128 : (J - j0 + 1) * 128]
                            rhs = v_of(Sb, kt)
                            st_f = idx == 0
                            sp_f = idx == len(chunks) - 1
                            nc.tensor.matmul(
                                o_ps[:, 0:512], lhsT, rhs[0], start=st_f, stop=sp_f
                            )
                            nc.tensor.matmul(
                                o_ps[:, 512:1024], lhsT, rhs[1], start=st_f, stop=sp_f
                            )
                            nc.tensor.matmul(
                                o_ps[:, 1024:1025],
                                lhsT,
                                sb_ones[:],
                                start=st_f,
                                stop=sp_f,
                            )
                        if first:
                            nc.vector.tensor_copy(o_acc[J][:], o_ps[:, 0 : D + 1])
                        else:
                            nc.vector.tensor_add(
                                o_acc[J][:], o_acc[J][:], o_ps[:, 0 : D + 1]
                            )
                        if last:
                            rs = fin.tile([128, 1], F32, tag="rs", name="rs")
                            nc.vector.reciprocal(rs[:], o_acc[J][:, D : D + 1])
                            outt = fin.tile([128, D], F32, tag="outt", name="outt")
                            nc.vector.tensor_scalar_mul(
                                outt[:], o_acc[J][:, 0:D], rs[:]
                            )
                            nc.sync.dma_start(
                                out[J * 128 : (J + 1) * 128, :], outt[:]
                            )

                # ---- ST half 0 (kta/ktb reads on sync) ----
                k0p_cm = tc.tile_pool(name="k0p", bufs=16)
                k0p = k0p_cm.__enter__()
                psst_cm = tc.tile_pool(name="psst0", bufs=4, space="PSUM")
                psst = psst_cm.__enter__()
                for piece, gk in ((0, g_ka), (1, g_kb)):
                    for Sb in range(NC):
                        kt_t = k0p.tile(
                            [128, NCH * 256], BF16, tag="kt", name=f"kt{piece}_{Sb}"
                        )
                        nc.sync.dma_start(
                            kt_t.rearrange("p (c k) -> p c k", c=NCH),
                            gk[Sb].rearrange("(c p) k -> p c k", p=128),
                        )
                        for k2 in range(2):
                            kt = piece * 2 + k2
                            st_groups(
                                psst,
                                Sb,
                                kt,
                                lambda c, kt_t=kt_t, k2=k2: kt_t[
                                    :, c * 256 + k2 * 128 : c * 256 + k2 * 128 + 128
                                ],
                            )
                psst_cm.__exit__(None, None, None)
                k0p_cm.__exit__(None, None, None)

                # k1p reuses k0p's SBUF (its readers finished at ST-h0 end),
                # so the kt1 prefetches issued here stall on nothing but the
                # K-h1 AllGather itself.
                k1p_cm = tc.tile_pool(name="k1p", bufs=4)
                k1p = k1p_cm.__enter__()
                kt1_ts = {}
                for Sb in range(NC):
                    kt_t = k1p.tile([128, NCH * 512], BF16, tag="kt", name=f"kt1_{Sb}")
                    nc.sync.dma_start(
                        kt_t.rearrange("p (c k) -> p c k", c=NCH),
                        g_k1[Sb].rearrange("(c p) k -> p c k", p=128),
                    )
                    kt1_ts[Sb] = kt_t

                # ---- PV half 0 (va/vb reads on gpsimd, behind triggers) ----
                v0p_cm = tc.tile_pool(name="v0p", bufs=16)
                v0p = v0p_cm.__enter__()
                pso_cm = tc.tile_pool(name="pso0", bufs=2, space="PSUM")
                pso = pso_cm.__enter__()
                v_ts = {}
                for piece, gv in ((0, g_va), (1, g_vb)):
                    for Sb in range(NC):
                        v_t = v0p.tile(
                            [128, 2 * D], BF16, tag="v", name=f"v{piece}_{Sb}"
                        )
                        nc.gpsimd.dma_start(
                            v_t.rearrange("p (c d) -> p c d", c=2),
                            gv[Sb].rearrange("(c p) d -> p c d", p=128),
                        )
                        v_ts[(piece, Sb)] = v_t

                def v_of0(piece):
                    def f(Sb, kt):
                        k2 = kt - piece * 2
                        v_t = v_ts[(piece, Sb)]
                        return (
                            v_t[:, k2 * D : k2 * D + 512],
                            v_t[:, k2 * D + 512 : k2 * D + 1024],
                        )

                    return f

                pv_pass(pso, v_of0(0), (0, 1), first=True, last=False)
                pv_pass(pso, v_of0(1), (2, 3), first=False, last=False)
                pso_cm.__exit__(None, None, None)
                v0p_cm.__exit__(None, None, None)

                # v1p reuses v0p's SBUF (readers done at PV-h0 end).
                v1p_cm = tc.tile_pool(name="v1p", bufs=8)
                v1p = v1p_cm.__enter__()
                v1_ts = {}
                for Sb in range(NC):
                    v_t = v1p.tile([128, 4 * D], BF16, tag="v", name=f"v1_{Sb}")
                    nc.gpsimd.dma_start(
                        v_t.rearrange("p (c d) -> p c d", c=4),
                        g_v1[Sb].rearrange("(c p) d -> p c d", p=128),
                    )
                    v1_ts[Sb] = v_t

                # ---- ST half 1 ----
                psst_cm = tc.tile_pool(name="psst1", bufs=4, space="PSUM")
                psst = psst_cm.__enter__()
                for Sb in range(NC):
                    kt_t = kt1_ts[Sb]
                    for k4 in range(4):
                        kt = 4 + k4
                        st_groups(
                            psst,
                            Sb,
                            kt,
                            lambda c, kt_t=kt_t, k4=k4: kt_t[
                                :, c * 512 + k4 * 128 : c * 512 + k4 * 128 + 128
                            ],
                        )
                psst_cm.__exit__(None, None, None)

                # ---- PV half 1 + finalize ----
                pso_cm = tc.tile_pool(name="pso1", bufs=2, space="PSUM")
                pso = pso_cm.__enter__()

                def v_of1(Sb, kt):
                    k4 = kt - 4
                    v_t = v1_ts[Sb]
                    return (
                        v_t[:, k4 * D : k4 * D + 512],
                        v_t[:, k4 * D + 512 : k4 * D + 1024],
                    )

                pv_pass(pso, v_of1, (4, 5, 6, 7), first=False, last=True)
                pso_cm.__exit__(None, None, None)
                v1p_cm.__exit__(None, None, None)
                k1p_cm.__exit__(None, None, None)

    nc.compile()
    _cache["nc"] = nc
    return nc


def _make_in_maps(inputs, w_query, w_key, w_value):
    bf = ml_dtypes.bfloat16
    xt = np.ascontiguousarray(inputs.T.astype(np.float32))  # [D, S]
    # Wq absorbed into the key path: scores = x_k^T (Wk^T Wq) x_q
    wkT = np.ascontiguousarray(
        w_key.T.astype(np.float32) @ w_query.astype(np.float32)
    ).astype(bf)
    wvT = np.ascontiguousarray(w_value.T).astype(bf)

    kt_off = np.arange(8)[:, None, None] * 128 + np.arange(128)[None, :, None]
    in_maps = []
    for i in range(NC):
        xkv = np.ascontiguousarray(xt[:, i * QPC : (i + 1) * QPC]).astype(bf)
        xq = np.ascontiguousarray(xt[:, i::NC]).astype(bf)
        q_off = np.arange(128)[None, None, :] * 8 + i
        m = (kt_off <= q_off).astype(np.float32).astype(bf)  # [8,128,128]
        in_maps.append(
            {
                "xt_kv": xkv,
                "xt_q": xq,
                "wkT": wkT,
                "wvT": wvT,
                "masks": np.ascontiguousarray(m),
            }
        )
    return in_maps


def run(inputs, w_query, w_key, w_value, trace=False):
    nc = _build()
    in_maps = _make_in_maps(inputs, w_query, w_key, w_value)
    res = bass_utils.run_bass_kernel_spmd(
        nc, in_maps, core_ids=list(range(NC)), trace=trace
    )
    full = np.empty((S, D), dtype=np.float32)
    for i in range(NC):
        full[i::NC] = res.results[i]["out"]
    return full, res


def kernel(inputs, w_query, w_key, w_value):
    inputs = np.asarray(inputs, dtype=np.float32)
    w_query = np.asarray(w_query, dtype=np.float32)
    w_key = np.asarray(w_key, dtype=np.float32)
    w_value = np.asarray(w_value, dtype=np.float32)
    full, _ = run(inputs, w_query, w_key, w_value, trace=False)
    return full
